# revision 1
# baseline (speedup 1.0000x reference)
"""GAT layer kernel for Trainium2, 8-core data-parallel over batch.

Math (per batch b, head h):
    h = x @ W                              [N, H*HD]
    s_n = <h[n, h*HD:(h+1)*HD], a_src[h]>  t_n likewise with a_dst
    A[j, i] = exp(leakyrelu(s_i + t_j, 0.2))
            = max(e^{t_j} * e^{s_i}, e^{0.2 t_j} * e^{0.2 s_i})   (exact identity)
    out[i]  = (sum_j A[j, i] * h_j) / (sum_j A[j, i])
No row-max subtraction is needed: max(s_i + t_j) ~ 51 for these inputs, and
exp(51) ~ 1.4e22 is far below the fp32/bf16 overflow threshold; softmax ratios
are scale-invariant so unnormalized exponentials are numerically fine.

Per core (= one batch element):
  - hT/h_node/s/t via small float32r matmuls (fp32 is 1/4 rate on PE)
  - A tiles [j, i] (j on partitions) built per (head, j-tile) by one of:
      'act': E = Prelu(S_bcast + t_col, alpha=0.2) ; A = Exp(E)      (2 ACT ops)
      'dve': R2 = Es02_bcast * Et02_col ; A = max(Es_bcast * Et_col, R2)
             (tensor_scalar + scalar_tensor_tensor, bf16, 2 DVE ops)
    The row-broadcast tensors come from DMA partition-broadcasts split over
    three DMA lanes (sync / gpsimd / tensor queues).
  - out^T[(h,d)+Z, i] accumulated in PSUM: lhsT = [h_node | ones] block, rhs = A
  - normalize by the Z row: fold Z into [128, NT] columns via DRAM (the DVE
    reciprocal is ~8 cyc/elem serial along the free dim, so a [1, N] row costs
    8.5us but [128, NT] is ~0.1us), unfold, K=1-matmul-broadcast, multiply.
  - engines have strict-FIFO instruction streams, so every op that waits on a
    whole head's matmul output is emitted with a one-head lag (or after the
    bulk loop) to avoid stalling the producers.
  - host transposes out^T back to node-major when unsharding.
"""

import numpy as np

B, N, IN_F, OUT_F, H = 8, 1024, 128, 128, 4
HD = OUT_F // H  # 32
NEG = 0.2
N_CORES = 8
NT = N // 128  # 8 node tiles


def _default_schemes():
    # Interleave within each head so ScalarE and VectorE stay busy together.
    # interleave within each head so ScalarE and VectorE stay busy together
    # AND the PE consumes tiles at a steady mixed pace; 14 act / 18 dve.
    sch = {(h, jt): ("act" if jt in (0, 2, 4) else "dve")
           for h in range(H) for jt in range(NT)}
    sch[(0, 6)] = "act"
    sch[(2, 6)] = "act"
    return sch


SCHEMES = _default_schemes()

A_DT = "bfloat16"  # dtype of the attention tiles + matmul weights

_CACHE = {}


def _build_nc():
    import concourse.bacc as bacc
    import concourse.tile as tile
    from concourse import mybir

    f32 = mybir.dt.float32
    f16 = mybir.dt.float16
    f32r = mybir.dt.float32r
    adt = getattr(mybir.dt, A_DT)
    AF = mybir.ActivationFunctionType
    ALU = mybir.AluOpType

    nc = bacc.Bacc("TRN2", target_bir_lowering=False, debug=False,
                   num_devices=N_CORES)

    xT = nc.declare_dram_parameter("xT", [IN_F, N], f32, isOutput=False)
    Wd = nc.declare_dram_parameter("W", [IN_F, OUT_F], f32, isOutput=False)
    Wa = nc.declare_dram_parameter("Wa", [IN_F, 2 * H], f32, isOutput=False)
    ind4_d = nc.declare_dram_parameter("ind4", [H, OUT_F], f32, isOutput=False)
    outT = nc.declare_dram_parameter("outT", [OUT_F, N], f32, isOutput=True)

    st_dram = nc.dram_tensor("st_scratch", [2 * H, N], f32)
    s16_dram = nc.dram_tensor("s16_scratch", [H, N], f16)
    es_dram = nc.dram_tensor("es_scratch", [H, N], adt)
    es02_dram = nc.dram_tensor("es02_scratch", [H, N], adt)
    z_dram = nc.dram_tensor("z_scratch", [H, N], f32)
    rz_dram = nc.dram_tensor("rz_scratch", [H, N], f32)

    with tile.TileContext(nc) as tc:
      with (
        tc.tile_pool(name="const", bufs=1) as cpool,
        tc.tile_pool(name="etile", bufs=4) as epool,
        tc.tile_pool(name="atile", bufs=16) as apool,
        tc.tile_pool(name="r2tile", bufs=4) as r2pool,
        tc.tile_pool(name="otile", bufs=1) as opool,
        tc.tile_pool(name="rztile", bufs=1) as rzpool,
      ):
        with tc.tile_pool(name="ps_pre", bufs=1, space="PSUM") as pspre:
            # ---- load inputs ----
            xT_sb = cpool.tile([IN_F, N], f32, tag="xT")
            nc.sync.dma_start(out=xT_sb[:, 0:512], in_=xT[:, 0:512])
            nc.gpsimd.dma_start(out=xT_sb[:, 512:N], in_=xT[:, 512:N])
            W_sb = cpool.tile([IN_F, OUT_F], f32, tag="W")
            nc.sync.dma_start(out=W_sb, in_=Wd[:])
            Wa_sb = cpool.tile([IN_F, 2 * H], f32, tag="Wa")
            nc.sync.dma_start(out=Wa_sb, in_=Wa[:])

            # fp32 matmul is 1/4 rate; float32r streams 1 col/cycle at N>=256
            # but needs explicitly rounded inputs (a convert copy).
            xTr = cpool.tile([IN_F, N], f32r, tag="xTr")
            nc.vector.tensor_copy(out=xTr, in_=xT_sb)
            Wr = cpool.tile([IN_F, OUT_F], f32r, tag="Wr")
            nc.vector.tensor_copy(out=Wr, in_=W_sb)
            War = cpool.tile([IN_F, 2 * H], f32r, tag="War")
            nc.vector.tensor_copy(out=War, in_=Wa_sb)

            # ---- st rows = (W @ a_ext)^T @ xT -> [2H, N]; the host
            # precomputes Wa = W @ a_ext so s/t skip the hT dependency ----
            st_ps = pspre.tile([2 * H, N], f32, tag="st")
            for c in range(2):
                nc.tensor.matmul(st_ps[:, 512 * c:512 * (c + 1)], War,
                                 xTr[:, 512 * c:512 * (c + 1)],
                                 start=True, stop=True)
            st_sb = cpool.tile([2 * H, N], f32, tag="st_sb")
            nc.vector.tensor_copy(out=st_sb, in_=st_ps)

            # ---- t columns via DRAM round trip ----
            nc.sync.dma_start(out=st_dram[:], in_=st_sb)
            # tc_all[p, h*NT+jt] = t_col for head h, j-tile jt
            tc_all = cpool.tile([128, H * NT], f32, tag="tc")
            nc.gpsimd.dma_start(
                out=tc_all,
                in_=st_dram[:].rearrange("h (jt p) -> p (h jt)", p=128)[
                    :, H * NT:],
            )

            # fp16 s rows for the ACT-path broadcasts (half the DMA bytes;
            # |s| < 40 so fp16 is safe, and its tiny rounding is a per-column
            # scale that cancels in the softmax). These and the exp'd s rows
            # are emitted before anything that needs the tc_all DRAM hop so
            # the broadcast chain starts as early as possible.
            s16_rows = cpool.tile([H, N], f16, tag="s16")
            nc.vector.tensor_copy(out=s16_rows, in_=st_sb[0:H, :])
            nc.sync.dma_start(out=s16_dram[:], in_=s16_rows)

            # exp'd s rows (bf16) for the DVE path broadcasts
            es_rows = cpool.tile([H, N], adt, tag="es_rows")
            nc.scalar.activation(out=es_rows, in_=st_sb[0:H, :], func=AF.Exp)
            nc.gpsimd.dma_start(out=es_dram[:], in_=es_rows)
            es02_rows = cpool.tile([H, N], adt, tag="es02_rows")
            nc.scalar.activation(out=es02_rows, in_=st_sb[0:H, :], func=AF.Exp,
                                 scale=NEG)
            nc.sync.dma_start(out=es02_dram[:], in_=es02_rows)

            # exp'd per-partition scalars for the DVE path
            etc = cpool.tile([128, H * NT], f32, tag="etc")
            nc.scalar.activation(out=etc, in_=tc_all, func=AF.Exp)
            etc02 = cpool.tile([128, H * NT], f32, tag="etc02")
            nc.scalar.activation(out=etc02, in_=tc_all, func=AF.Exp, scale=NEG)

            # indicator weights: ind[k, 32h+d] = (k == h) — used to broadcast
            # row h of a [4, N] tile across 32 output partitions via K=4 matmul
            ind4_f = cpool.tile([H, OUT_F], f32, tag="ind4f")
            nc.sync.dma_start(out=ind4_f, in_=ind4_d[:])
            ind4 = cpool.tile([H, OUT_F], f32r, tag="ind4")
            nc.vector.tensor_copy(out=ind4, in_=ind4_f)

            # ---- weight tiles: wt[:, 132jt+33h : +32] = h_node + a 1s col ----
            hn_ps = pspre.tile([128, N], f32, tag="hn")
            for jt in range(NT):
                nc.tensor.matmul(hn_ps[:, 128 * jt:128 * (jt + 1)],
                                 xTr[:, 128 * jt:128 * (jt + 1)], Wr,
                                 start=True, stop=True)
            wt_all = cpool.tile([128, NT * 33 * H], adt, tag="wt")
            wt_v = wt_all[:].rearrange("p (jt h c) -> p jt h c", h=H, c=33)
            nc.vector.tensor_copy(
                out=wt_v[:, :, :, 0:32],
                in_=hn_ps[:].rearrange("p (jt h c) -> p jt h c", h=H, c=32))
            nc.vector.memset(wt_v[:, :, :, 32:33], 1.0)
            wts = [wt_all[:, 132 * jt:132 * (jt + 1)] for jt in range(NT)]

        # ---- broadcast tiles per head, spread over three DMA lanes ----
        s_bcast, es_b, es02_b = {}, {}, {}
        for h in range(H):
            sb = cpool.tile([128, N], f16, tag=f"sb{h}")
            nc.sync.dma_start(
                out=sb, in_=s16_dram[h:h + 1, :].to_broadcast([128, N]))
            s_bcast[h] = sb
            eb2 = cpool.tile([128, N], adt, tag=f"es02b{h}")
            nc.sync.dma_start(
                out=eb2, in_=es02_dram[h:h + 1, :].to_broadcast([128, N]))
            es02_b[h] = eb2
            eb = cpool.tile([128, N], adt, tag=f"esb{h}")
            nc.gpsimd.dma_start(
                out=eb, in_=es_dram[h:h + 1, :].to_broadcast([128, N]))
            es_b[h] = eb

        # ---- main loop: oh bufs=4 keeps all four heads' accumulators
        # resident (8 PSUM banks) so no matmul ever waits on a slot release
        with tc.tile_pool(name="ps_main", bufs=4, space="PSUM") as psmain:
            ohs = [None] * H
            for h in range(H):
                oh = psmain.tile([33, N], f32, tag="oh")
                ohs[h] = oh
                for jt in range(NT):
                    idx = h * NT + jt
                    if SCHEMES[(h, jt)] == "act":
                        e_t = epool.tile([128, N], f32, tag="et")
                        nc.scalar.activation(out=e_t, in_=s_bcast[h],
                                             func=AF.Prelu,
                                             bias=tc_all[:, idx:idx + 1],
                                             scale=1.0, alpha=NEG)
                        a_t = apool.tile([128, N], adt, tag="at")
                        nc.scalar.activation(out=a_t, in_=e_t, func=AF.Exp)
                    else:
                        r2 = r2pool.tile([128, N], adt, tag="r2")
                        nc.vector.tensor_scalar_mul(
                            out=r2, in0=es02_b[h],
                            scalar1=etc02[:, idx:idx + 1])
                        a_t = apool.tile([128, N], adt, tag="at")
                        nc.vector.scalar_tensor_tensor(
                            out=a_t, in0=es_b[h], scalar=etc[:, idx:idx + 1],
                            in1=r2, op0=ALU.mult, op1=ALU.max)
                    for c in range(2):
                        nc.tensor.matmul(
                            oh[:, 512 * c:512 * (c + 1)],
                            wts[jt][:, 33 * h:33 * (h + 1)],
                            a_t[:, 512 * c:512 * (c + 1)],
                            start=(jt == 0), stop=(jt == NT - 1))
            # Scheduler-time floor: without it the Tile scheduler's cost
            # model (which underestimates DMA latency) interleaves these
            # tail ops ahead of bulk ops in the strict-FIFO engine streams,
            # stalling the producers for tens of us.
            tail_ctx = tc.tile_wait_until(0.2)
            tail_ctx.__enter__()
            # ---- deferred normalize tail. 1/Z via integer-magic seed +
            # 2 Newton-Raphson iterations on DVE, batched [H, N] for all
            # heads: ~7us of engine ops with a single small-DMA stage (the
            # Z-row assembly) instead of 4 DMA round-trips per head whose
            # ~5us/hop latency dominated earlier revisions. Z in [6e-6, 1e25]
            # is comfortably inside the magic-constant seed's valid range.
            ocps = []
            for h in range(H):
                ocp = opool.tile([33, N], f32, tag=f"ocp{h}")
                if h < 2:
                    nc.scalar.copy(out=ocp, in_=ohs[h])
                else:
                    nc.vector.tensor_copy(out=ocp, in_=ohs[h])
                ocps.append(ocp)
            # assemble Z rows in one [H, N] tile (4 small on-chip DMAs),
            # then 1/Z = int-magic seed + one Newton iteration (~0.3% seed^2
            # error, far inside the tolerance), f32r-rounded on the last op
            z4 = rzpool.tile([H, N], f32, tag="z4")
            for h in range(H):
                nc.sync.dma_start(out=z4[h:h + 1, :], in_=ocps[h][32:33, :])
            i32 = mybir.dt.int32
            ynot = rzpool.tile([H, N], f32, tag="ynot")
            nc.vector.tensor_scalar(
                out=ynot[:].bitcast(i32), in0=z4[:].bitcast(i32),
                scalar1=0xFFFFFFFF - (1 << 32), scalar2=None,
                op0=ALU.bitwise_xor)
            y = rzpool.tile([H, N], f32, tag="y")
            nc.vector.tensor_scalar(
                out=y[:].bitcast(i32), in0=ynot[:].bitcast(i32),
                scalar1=0x7EF311C4, scalar2=None, op0=ALU.add)
            m = rzpool.tile([H, N], f32, tag="nr_m")
            nc.vector.tensor_tensor(out=m, in0=z4, in1=y, op=ALU.mult)
            s2 = rzpool.tile([H, N], f32, tag="nr_s")
            nc.vector.tensor_scalar(out=s2, in0=m, scalar1=2.0,
                                    scalar2=-1.0, op0=ALU.subtract,
                                    op1=ALU.mult)
            rz4 = rzpool.tile([H, N], f32r, tag="rz4")
            nc.vector.tensor_tensor(out=rz4, in0=y, in1=s2, op=ALU.mult)
        with tc.tile_pool(name="ps_norm", bufs=2, space="PSUM") as psnorm:
            rzbs = []
            for h in range(H):
                rzb = psnorm.tile([HD, N], f32, tag="rzb")
                for c in range(2):
                    nc.tensor.matmul(rzb[:, 512 * c:512 * (c + 1)],
                                     ind4[:, HD * h:HD * (h + 1)],
                                     rz4[:, 512 * c:512 * (c + 1)],
                                     start=True, stop=True)
                rzbs.append(rzb)
                o_sb = opool.tile([HD, N], f32, tag=f"osb{h}")
                nc.vector.tensor_tensor(out=o_sb, in0=ocps[h][0:HD, :],
                                        in1=rzb, op=ALU.mult)
                nc.sync.dma_start(out=outT[HD * h:HD * (h + 1), :],
                                  in_=o_sb)
            tail_ctx.__exit__(None, None, None)

    nc.compile()
    return nc


def _get_nc():
    if "nc" not in _CACHE:
        _CACHE["nc"] = _build_nc()
    return _CACHE["nc"]


def kernel(x, W, a_src, a_dst):
    from concourse.bass_utils import run_bass_kernel_spmd

    x = np.asarray(x, dtype=np.float32)
    W = np.asarray(W, dtype=np.float32)
    a_src = np.asarray(a_src, dtype=np.float32)
    a_dst = np.asarray(a_dst, dtype=np.float32)

    a_ext = np.zeros((OUT_F, 2 * H), np.float32)
    ind4 = np.zeros((H, OUT_F), np.float32)
    for h in range(H):
        a_ext[h * HD:(h + 1) * HD, h] = a_src[h]
        a_ext[h * HD:(h + 1) * HD, H + h] = a_dst[h]
        ind4[h, h * HD:(h + 1) * HD] = 1.0
    Wa = W @ a_ext

    nc = _get_nc()
    in_maps = [
        {"xT": np.ascontiguousarray(x[c].T), "W": W, "Wa": Wa,
         "ind4": ind4}
        for c in range(N_CORES)
    ]
    res = run_bass_kernel_spmd(nc, in_maps, core_ids=list(range(N_CORES)))
    out = np.stack([res.results[c]["outT"].T for c in range(N_CORES)], axis=0)
    return np.ascontiguousarray(out, dtype=np.float32)



# revision 5
# speedup vs baseline: 1.4844x; 1.4844x over previous
"""GAT layer kernel for Trainium2, 8-core data-parallel over batch.

Math (per batch b, head h):
    h = x @ W                              [N, H*HD]
    s_n = <h[n, h*HD:(h+1)*HD], a_src[h]>  t_n likewise with a_dst
    A[j, i] = exp(leakyrelu(s_i + t_j, 0.2))
    out[i]  = (sum_j A[j, i] * h_j) / (sum_j A[j, i])

Key identity: softmax columns are scale-invariant, so drop the e^{s_i}
column factor entirely:
    A'[j, i] = A[j, i] * e^{-s_i} = max(e^{t_j}, e^{0.2 t_j} * u_i),
    u_i = e^{-0.8 s_i}
Both numerator and Z pick up the same e^{-s_i}, which cancels in the
division. Each [128, N] attention tile is then ONE vector tensor_scalar
op (in0 = broadcast u row, two per-partition scalar columns e^{t_j} /
e^{0.2 t_j}, ops mult+max) running in the DVE's 4x packed mode -- vs the
2 ACT / 2 DVE ops per tile of the direct formulation -- and only H=4
broadcast tiles (1 MB) of DMA instead of 12 (3 MB).

Per core (= one batch element):
  - st rows = (W @ a_ext)^T @ xT via one small f32r matmul (host
    precomputes Wa = W @ a_ext); u row = Exp(-0.8 s) on ACT; u broadcast
    [1,N] -> [128,N] via DRAM round trip on the DMA queues.
  - h_node blocks AND transposed t columns from the SAME per-j-tile
    matmul by extending its rhs to [W | Wa] (136 cols): out[128j, 0:128]
    = h_node, out[128j, 132:136] = t columns. No DRAM transpose gather.
  - main loop per (head, j-tile): one tensor_scalar -> A' tile (bf16),
    two 512-col matmuls accumulate [h_node | ones]^T @ A' into PSUM
    [33, N] (row 32 = Z').
  - tail: Z rows copied on idle ACT into z8 [8, 512] (rows = (h, half)),
    1/Z via int-magic seed + one Newton iteration (4 DVE ops, ~0.3%
    err^2), sign folded so nrz = -1/Z; one K=8 indicator matmul per
    512-half broadcasts nrz to all 128 (h,d) output rows; one fused
    (num * -1) * nrz multiply per half; DMA out. Numerator rows are
    copied PSUM->SBUF per head on ACT during the main loop so PSUM banks
    are free for the broadcast matmuls.
  - host transposes out^T back to node-major when unsharding.
"""

import numpy as np

B, N, IN_F, OUT_F, H = 8, 1024, 128, 128, 4
HD = OUT_F // H  # 32
NEG = 0.2
N_CORES = 8
NT = N // 128  # 8 node tiles

A_DT = "bfloat16"  # dtype of the attention tiles + matmul weights

_CACHE = {}


def _build_nc():
    import concourse.bacc as bacc
    import concourse.tile as tile
    from concourse import mybir

    f32 = mybir.dt.float32
    f32r = mybir.dt.float32r
    i32 = mybir.dt.int32
    adt = getattr(mybir.dt, A_DT)
    AF = mybir.ActivationFunctionType
    ALU = mybir.AluOpType

    nc = bacc.Bacc("TRN2", target_bir_lowering=False, debug=False,
                   num_devices=N_CORES)

    xT = nc.declare_dram_parameter("xT", [IN_F, N], f32, isOutput=False)
    Wd = nc.declare_dram_parameter("W", [IN_F, OUT_F], f32, isOutput=False)
    Wa = nc.declare_dram_parameter("Wa", [IN_F, 2 * H], f32, isOutput=False)
    ind8_d = nc.declare_dram_parameter("ind8", [2 * H, 2 * OUT_F], f32,
                                       isOutput=False)
    outT = nc.declare_dram_parameter("outT", [OUT_F, N], f32, isOutput=True)

    u_dram = nc.dram_tensor("u_scratch", [H, N], adt)

    with tile.TileContext(nc) as tc:
      with (
        tc.tile_pool(name="const", bufs=1) as cpool,
        tc.tile_pool(name="atile", bufs=12) as apool,
        tc.tile_pool(name="tail", bufs=1) as tpool,
      ):
        with tc.tile_pool(name="ps_pre", bufs=1, space="PSUM") as pspre:
            # ---- load inputs ----
            xT_sb = cpool.tile([IN_F, N], f32, tag="xT")
            nc.sync.dma_start(out=xT_sb[:, 0:512], in_=xT[:, 0:512])
            nc.gpsimd.dma_start(out=xT_sb[:, 512:N], in_=xT[:, 512:N])
            # [W | Wa] so one matmul per j-tile yields h_node + t columns
            WW = cpool.tile([IN_F, OUT_F + 2 * H], f32, tag="WW")
            nc.sync.dma_start(out=WW[:, 0:OUT_F], in_=Wd[:])
            nc.sync.dma_start(out=WW[:, OUT_F:], in_=Wa[:])
            ind8_f = cpool.tile([2 * H, 2 * OUT_F], f32, tag="ind8f")
            nc.gpsimd.dma_start(out=ind8_f, in_=ind8_d[:])

            # f32r casts for the full-rate st matmul (fp32 is 1/4 rate)
            xTr = cpool.tile([IN_F, N], f32r, tag="xTr")
            nc.vector.tensor_copy(out=xTr, in_=xT_sb)
            War4 = cpool.tile([IN_F, H], f32r, tag="War4")
            nc.vector.tensor_copy(out=War4, in_=WW[:, OUT_F:OUT_F + H])
            ind8 = cpool.tile([2 * H, 2 * OUT_F], f32r, tag="ind8")
            nc.vector.tensor_copy(out=ind8, in_=ind8_f)

            # ---- s rows -> u = exp(-0.8 s), broadcast via DRAM ----
            st_ps = pspre.tile([H, N], f32, tag="st")
            for c in range(2):
                nc.tensor.matmul(st_ps[:, 512 * c:512 * (c + 1)], War4,
                                 xTr[:, 512 * c:512 * (c + 1)],
                                 start=True, stop=True)
            u_rows = cpool.tile([H, N], adt, tag="u_rows")
            nc.scalar.activation(out=u_rows, in_=st_ps, func=AF.Exp,
                                 scale=-0.8)
            nc.sync.dma_start(out=u_dram[:], in_=u_rows)
            u_b = []
            for h in range(H):
                ub = cpool.tile([128, N], adt, tag=f"ub{h}")
                eng = nc.sync if h % 2 == 0 else nc.gpsimd
                eng.dma_start(
                    out=ub, in_=u_dram[h:h + 1, :].to_broadcast([128, N]))
                u_b.append(ub)

            # ---- h_node blocks + t columns: one matmul per j-tile,
            # rhs = [W | Wa] (fp32: 136 cols is below the f32r full-rate
            # threshold anyway, and this skips the rhs cast) ----
            CW = 256  # per-jt column pitch in PSUM (pads to bank size)
            hnst = pspre.tile([128, NT * CW], f32, tag="hnst")
            for jt in range(NT):
                nc.tensor.matmul(
                    hnst[:, CW * jt:CW * jt + OUT_F + 2 * H],
                    xT_sb[:, 128 * jt:128 * (jt + 1)],
                    WW, start=True, stop=True)

            # t scalar columns: tc_all[p, h*NT+jt] = t[h, 128*jt+p]
            tc_all = cpool.tile([128, H * NT], f32, tag="tc")
            nc.vector.tensor_copy(
                out=tc_all[:].rearrange("p (h jt) -> p jt h", jt=NT),
                in_=hnst[:].rearrange("p (jt c) -> p jt c", c=CW)[
                    :, :, OUT_F + H:OUT_F + 2 * H])
            etc = cpool.tile([128, H * NT], f32, tag="etc")
            nc.scalar.activation(out=etc, in_=tc_all, func=AF.Exp)
            etc02 = cpool.tile([128, H * NT], f32, tag="etc02")
            nc.scalar.activation(out=etc02, in_=tc_all, func=AF.Exp,
                                 scale=NEG)

            # weight tiles: wt[:, 132jt+33h : +32] = h_node block + 1s col
            wt_all = cpool.tile([128, NT * 33 * H], adt, tag="wt")
            wt_v = wt_all[:].rearrange("p (jt h c) -> p jt h c", h=H, c=33)
            nc.scalar.copy(
                out=wt_v[:, :, :, 0:32],
                in_=hnst[:].rearrange("p (jt c) -> p jt c", c=CW)[
                    :, :, 0:OUT_F].rearrange("p jt (h d) -> p jt h d", d=HD))
            nc.gpsimd.memset(wt_v[:, :, :, 32:33], 1.0)
            wts = [wt_all[:, 132 * jt:132 * (jt + 1)] for jt in range(NT)]

        # num4[32h+d, i] = unnormalized out rows; z8[2h+c, i'] = Z halves
        num4 = tpool.tile([128, N], f32, tag="num4")
        z8 = tpool.tile([2 * H, 512], f32, tag="z8")

        # ---- main loop: one tensor_scalar + two matmuls per (h, jt) ----
        with tc.tile_pool(name="ps_main", bufs=4, space="PSUM") as psmain:
            for h in range(H):
                oh = psmain.tile([33, N], f32, tag="oh")
                for jt in range(NT):
                    idx = h * NT + jt
                    a_t = apool.tile([128, N], adt, tag="at")
                    nc.vector.tensor_scalar(
                        out=a_t, in0=u_b[h],
                        scalar1=etc02[:, idx:idx + 1],
                        scalar2=etc[:, idx:idx + 1],
                        op0=ALU.mult, op1=ALU.max)
                    for c in range(2):
                        nc.tensor.matmul(
                            oh[:, 512 * c:512 * (c + 1)],
                            wts[jt][:, 33 * h:33 * (h + 1)],
                            a_t[:, 512 * c:512 * (c + 1)],
                            start=(jt == 0), stop=(jt == NT - 1))
                # per-head epilogue, pipelined with the next heads' bulk
                # work: Z row folds [1,1024]->[2,512] via DMA (engines
                # cannot write partition offsets that aren't 32-aligned),
                # numerator rows via the otherwise-idle ACT engine.
                zrow = tpool.tile([1, N], f32, tag=f"zrow{h}")
                nc.scalar.copy(out=zrow, in_=oh[32:33, :])
                eng = nc.sync if h % 2 == 0 else nc.gpsimd
                for c in range(2):
                    eng.dma_start(
                        out=z8[2 * h + c:2 * h + c + 1, :],
                        in_=zrow[:, 512 * c:512 * (c + 1)])
                nc.scalar.copy(out=num4[HD * h:HD * (h + 1), :],
                               in_=oh[0:32, :])

        # ---- tail: nrz = -1/Z via int-magic seed + one Newton step ----
        ynot = tpool.tile([2 * H, 512], f32, tag="ynot")
        nc.vector.tensor_scalar(
            out=ynot[:].bitcast(i32), in0=z8[:].bitcast(i32),
            scalar1=0xFFFFFFFF - (1 << 32), scalar2=None,
            op0=ALU.bitwise_xor)
        y = tpool.tile([2 * H, 512], f32, tag="y")
        nc.vector.tensor_scalar(
            out=y[:].bitcast(i32), in0=ynot[:].bitcast(i32),
            scalar1=0x7EF311C4, scalar2=None, op0=ALU.add)
        m = tpool.tile([2 * H, 512], f32, tag="nr_m")
        nc.vector.tensor_tensor(out=m, in0=z8, in1=y, op=ALU.mult)
        nrz = tpool.tile([2 * H, 512], f32r, tag="nrz")
        nc.vector.scalar_tensor_tensor(out=nrz, in0=m, scalar=2.0,
                                       in1=y, op0=ALU.subtract,
                                       op1=ALU.mult)  # (m-2)*y = -1/Z
        with tc.tile_pool(name="ps_norm", bufs=1, space="PSUM") as psnorm:
            for c in range(2):
                # rzb[32h+d, i'] = nrz[2h+c, i'] via K=8 indicator matmul
                rzb = psnorm.tile([128, 512], f32, tag=f"rzb{c}")
                nc.tensor.matmul(rzb[:, :], ind8[:, 128 * c:128 * (c + 1)],
                                 nrz[:, :], start=True, stop=True)
                o_sb = tpool.tile([128, 512], f32, tag=f"osb{c}")
                nc.vector.scalar_tensor_tensor(
                    out=o_sb, in0=num4[:, 512 * c:512 * (c + 1)],
                    scalar=-1.0, in1=rzb, op0=ALU.mult, op1=ALU.mult)
                eng = nc.sync if c == 0 else nc.gpsimd
                eng.dma_start(out=outT[:, 512 * c:512 * (c + 1)], in_=o_sb)

    nc.compile()
    return nc


def _get_nc():
    if "nc" not in _CACHE:
        _CACHE["nc"] = _build_nc()
    return _CACHE["nc"]


def make_in_maps(x, W, a_src, a_dst):
    a_ext = np.zeros((OUT_F, 2 * H), np.float32)
    for h in range(H):
        a_ext[h * HD:(h + 1) * HD, h] = a_src[h]
        a_ext[h * HD:(h + 1) * HD, H + h] = a_dst[h]
    Wa = W @ a_ext
    # ind8[k, 128c + p] = 1 iff k == 2*(p//32) + c (Z-row broadcast)
    ind8 = np.zeros((2 * H, 2 * OUT_F), np.float32)
    for c in range(2):
        for h in range(H):
            ind8[2 * h + c, 128 * c + HD * h:128 * c + HD * (h + 1)] = 1.0
    return [
        {"xT": np.ascontiguousarray(x[c].T), "W": W, "Wa": Wa, "ind8": ind8}
        for c in range(N_CORES)
    ]


def kernel(x, W, a_src, a_dst):
    from concourse.bass_utils import run_bass_kernel_spmd

    x = np.asarray(x, dtype=np.float32)
    W = np.asarray(W, dtype=np.float32)
    a_src = np.asarray(a_src, dtype=np.float32)
    a_dst = np.asarray(a_dst, dtype=np.float32)

    nc = _get_nc()
    in_maps = make_in_maps(x, W, a_src, a_dst)
    res = run_bass_kernel_spmd(nc, in_maps, core_ids=list(range(N_CORES)))
    out = np.stack([res.results[c]["outT"].T for c in range(N_CORES)], axis=0)
    return np.ascontiguousarray(out, dtype=np.float32)
